# revision 1
# baseline (speedup 1.0000x reference)
"""MoE router (AutonomousRouter) for TRN2, 8 NeuronCores.

Computes reference:
    act    = einsum('bnd,edc->bnec', x, W)          B,N,D,E,C = 4,2048,2048,8,512
    logits = ||act||_2 over c                       [B,N,E]
    probs  = softmax(logits, -1)
    top-2 routing with capacity 640 (priority = order within k-major (choice, token) sequence)
    out    = stack([dispatch, combine])             [2,B,N,E,640] fp32

Sharding: data-parallel over tokens; core i <- tokens [i*1024, (i+1)*1024) of the
flattened [8192] token axis (= batch b=i//2, half i%2). Weights replicated.

Phase A (device): bf16x2-split matmuls (fp32-grade logits at 3x bf16 rate) ->
  sum-of-squares -> top-2 via max8 on sumsq (monotone in logits, sub-ulp
  lower-index tie-break) -> softmax (ACT sqrt/exp) -> one-hots -> core-local
  exclusive cumsums per choice slot (PE triangular matmuls, exact integer fp32).
Host glue: combines per-core totals into cross-core priority offsets (64 scalars).
Phase B (device): per-(token,choice) one-hot rows (iota==slot)*{1,prob} built on
  DVE and indirect-scattered into the pre-zeroed dense outputs.
"""
import numpy as np

import concourse.bacc as bacc
import concourse.mybir as mybir
from concourse.tile import TileContext
from concourse.bass_utils import run_bass_kernel_spmd

P = 128          # partitions
B, N, D, E, C = 4, 2048, 2048, 8, 512
CAP = 640
NCORES = 8
TOK = (B * N) // NCORES     # tokens per core = 1024
NT = TOK // P               # token tiles per core = 8
KT = D // P                 # contraction tiles = 16

f32 = mybir.dt.float32

_cache = {}
LAST_IN_MAPS_A = None   # kept for test harness re-runs/profiling
LAST_IN_MAPS_B = None


def _build_phase_a():
    bf16 = mybir.dt.bfloat16
    nc = bacc.Bacc("TRN2", target_bir_lowering=False, debug=False, num_devices=NCORES)
    # x/w pre-split on host into bf16 hi+lo: x = xh + xl exactly to ~2^-17 rel.
    # 3 bf16 matmuls (hh, hl, lh) at full PE rate replace one 1/4-rate fp32
    # matmul; products are exact in fp32, PSUM accumulation identical.
    xTh = nc.dram_tensor("xTh", [D, TOK], bf16, kind="ExternalInput")
    xTl = nc.dram_tensor("xTl", [D, TOK], bf16, kind="ExternalInput")
    wh = nc.dram_tensor("wh", [E, D, C], bf16, kind="ExternalInput")
    wl = nc.dram_tensor("wl", [E, D, C], bf16, kind="ExternalInput")
    linc = nc.dram_tensor("linc", [P, P], f32, kind="ExternalInput")     # linc[k,m]=1 if k<=m
    ones_k1 = nc.dram_tensor("ones_k1", [1, P], f32, kind="ExternalInput")
    ones128 = nc.dram_tensor("ones128", [P, 1], f32, kind="ExternalInput")
    iota8 = nc.dram_tensor("iota8", [P, E], f32, kind="ExternalInput")
    ebias = nc.dram_tensor("ebias", [P, E], f32, kind="ExternalInput")
    probs_out = nc.dram_tensor("probs", [TOK, E], f32, kind="ExternalOutput")
    s0_out = nc.dram_tensor("s0", [TOK, E], f32, kind="ExternalOutput")
    s1_out = nc.dram_tensor("s1", [TOK, E], f32, kind="ExternalOutput")

    with TileContext(nc) as tc:
        with (
            tc.tile_pool(name="const", bufs=1) as cpool,
            tc.tile_pool(name="wbuf", bufs=2) as wpool,
            tc.tile_pool(name="work", bufs=3) as spool,
            tc.tile_pool(name="ss", bufs=1) as sspool,
            tc.tile_pool(name="psum", bufs=8, space="PSUM") as psum,
        ):
            # x^T hi/lo resident in variable k-chunk tiles; W per expert likewise
            # (double-buffered). DMAs are issued in consumption order and the
            # first chunk is a single k-block, so the first matmuls wait on
            # ~0.8MB instead of the full 12MB.
            CHUNKS = [1, 3, 4, 4, 4]           # k-blocks per chunk, sums to KT
            CH0 = [sum(CHUNKS[:i]) for i in range(len(CHUNKS))]  # chunk k-starts
            NCH = len(CHUNKS)

            def _x_chunk(dram, q, name):
                nk = CHUNKS[q]
                tile_ = cpool.tile([P, nk * TOK], bf16, tag=name, name=name)
                nc.sync.dma_start(
                    out=tile_[:].rearrange("p (k n) -> p k n", k=nk),
                    in_=dram.ap()[CH0[q] * P:(CH0[q] + nk) * P, :]
                        .rearrange("(k p) n -> p k n", p=P),
                )
                return tile_

            def _w_chunk(dram, e, q, tag, name):
                nk = CHUNKS[q]
                tile_ = wpool.tile([P, nk * C], bf16, tag=tag, name=name)
                nc.sync.dma_start(
                    out=tile_[:].rearrange("p (k c) -> p k c", k=nk),
                    in_=dram.ap()[e, CH0[q] * P:(CH0[q] + nk) * P, :]
                        .rearrange("(k p) c -> p k c", p=P),
                )
                return tile_

            def _w_expert(e):
                return (
                    [_w_chunk(wh, e, q, f"whq{q}", f"wh{e}_{q}") for q in range(NCH)],
                    [_w_chunk(wl, e, q, f"wlq{q}", f"wl{e}_{q}") for q in range(NCH)],
                )

            # consumption-order issue: W(e0,q0), x(q0), W(e0,q1), x(q1), ...
            wth0_q, wtl0_q = [], []
            xth_q, xtl_q = [], []
            for q in range(NCH):
                wth0_q.append(_w_chunk(wh, 0, q, f"whq{q}", f"wh0_{q}"))
                wtl0_q.append(_w_chunk(wl, 0, q, f"wlq{q}", f"wl0_{q}"))
                xth_q.append(_x_chunk(xTh, q, f"xthq{q}"))
                xtl_q.append(_x_chunk(xTl, q, f"xtlq{q}"))
            linc_sb = cpool.tile([P, P], f32, tag="linc")
            nc.sync.dma_start(out=linc_sb[:], in_=linc.ap()[:, :])
            onesk1_sb = cpool.tile([1, P], f32, tag="onesk1")
            nc.sync.dma_start(out=onesk1_sb[:], in_=ones_k1.ap()[:, :])
            ones128_sb = cpool.tile([P, 1], f32, tag="ones128")
            nc.sync.dma_start(out=ones128_sb[:], in_=ones128.ap()[:, :])
            iota8_sb = cpool.tile([P, E], f32, tag="iota8")
            nc.sync.dma_start(out=iota8_sb[:], in_=iota8.ap()[:, :])
            ebias_sb = cpool.tile([P, E], f32, tag="ebias")
            nc.sync.dma_start(out=ebias_sb[:], in_=ebias.ap()[:, :])
            offs = cpool.tile([1, 2 * E], f32, tag="offs")
            nc.vector.memset(offs[:], 0.0)

            # per-token-tile sum-of-squares accumulators [128, E]
            ss_tiles = [cpool.tile([P, E], f32, tag=f"ss{t}", name=f"ss{t}")
                        for t in range(NT)]

            # ---- matmul phase: for each expert, 8 token tiles x 16 k-tiles ----
            for e in range(E):
                if e == 0:
                    wth_q, wtl_q = wth0_q, wtl0_q
                else:
                    wth_q, wtl_q = _w_expert(e)
                for t in range(NT):
                    ps = psum.tile([P, C], f32, space="PSUM", tag="ps")
                    first = True
                    for k in range(KT):
                        q = max(i for i in range(NCH) if CH0[i] <= k)
                        kq = k - CH0[q]
                        xh_blk = xth_q[q][:, kq * TOK + t * P: kq * TOK + (t + 1) * P]
                        xl_blk = xtl_q[q][:, kq * TOK + t * P: kq * TOK + (t + 1) * P]
                        wh_blk = wth_q[q][:, kq * C:(kq + 1) * C]
                        wl_blk = wtl_q[q][:, kq * C:(kq + 1) * C]
                        for lhsT, rhs in ((xh_blk, wh_blk), (xh_blk, wl_blk), (xl_blk, wh_blk)):
                            nc.tensor.matmul(
                                ps[:], lhsT=lhsT, rhs=rhs,
                                start=first,
                                stop=(k == KT - 1 and rhs is wh_blk and lhsT is xl_blk),
                            )
                            first = False
                    sq = spool.tile([P, C], f32, tag="sq")
                    nc.scalar.activation(sq[:], ps[:], mybir.ActivationFunctionType.Square)
                    red8 = spool.tile([P, 8], f32, tag="red8")
                    nc.vector.tensor_reduce(
                        red8[:], sq[:].rearrange("p (g c) -> p g c", g=8),
                        axis=mybir.AxisListType.X, op=mybir.AluOpType.add,
                    )
                    nc.vector.tensor_reduce(
                        ss_tiles[t][:, e:e + 1], red8[:],
                        axis=mybir.AxisListType.X, op=mybir.AluOpType.add,
                    )

            # ---- routing phase (order matters for the offs chain: t ascending) ----
            for t in range(NT):
                ss = ss_tiles[t]
                # sub-ulp lower-index tie-break: selection on ss - e*1e-4 (~half a
                # logit ulp); softmax shift-invariance keeps probs exact.
                ssb = spool.tile([P, E], f32, tag="ssb")
                nc.vector.tensor_add(out=ssb[:], in0=ss[:], in1=ebias_sb[:])
                top8 = spool.tile([P, 8], f32, tag="top8")
                top8i = spool.tile([P, 8], mybir.dt.uint32, tag="top8i")
                nc.vector.max_with_indices(top8[:], top8i[:], ssb[:])
                idxf = spool.tile([P, 8], f32, tag="idxf")
                nc.vector.tensor_copy(out=idxf[:], in_=top8i[:])

                logits = spool.tile([P, E], f32, tag="logits")
                nc.scalar.activation(logits[:], ss[:], mybir.ActivationFunctionType.Sqrt)
                lmax = spool.tile([P, 1], f32, tag="lmax")
                nc.scalar.activation(lmax[:], top8[:, 0:1], mybir.ActivationFunctionType.Sqrt)
                neg_lmax = spool.tile([P, 1], f32, tag="neglmax")
                nc.vector.tensor_scalar_mul(neg_lmax[:], lmax[:], -1.0)
                expt = spool.tile([P, E], f32, tag="expt")
                nc.scalar.activation(expt[:], logits[:], mybir.ActivationFunctionType.Exp,
                                     bias=neg_lmax[:], scale=1.0)
                denom = spool.tile([P, 1], f32, tag="denom")
                nc.vector.tensor_reduce(denom[:], expt[:], axis=mybir.AxisListType.X,
                                        op=mybir.AluOpType.add)
                rden = spool.tile([P, 1], f32, tag="rden")
                nc.vector.reciprocal(rden[:], denom[:])
                probs = spool.tile([P, E], f32, tag="probs")
                nc.vector.tensor_scalar(probs[:], expt[:], rden[:, 0:1], None,
                                        op0=mybir.AluOpType.mult)
                nc.sync.dma_start(out=probs_out.ap()[t * P:(t + 1) * P, :], in_=probs[:])

                for kk, icol in ((0, 0), (1, 1)):
                    m = spool.tile([P, E], f32, tag=f"m{kk}")
                    nc.vector.tensor_scalar(m[:], iota8_sb[:], idxf[:, icol:icol + 1], None,
                                            op0=mybir.AluOpType.is_equal)
                    cum = psum.tile([P, E], f32, space="PSUM", tag="ps")
                    nc.tensor.matmul(cum[:], lhsT=linc_sb[:], rhs=m[:], start=True, stop=False)
                    nc.tensor.matmul(cum[:], lhsT=onesk1_sb[:], rhs=offs[:, kk * E:(kk + 1) * E],
                                     start=False, stop=True)
                    tot = psum.tile([1, E], f32, space="PSUM", tag="ps")
                    nc.tensor.matmul(tot[:], lhsT=ones128_sb[:], rhs=m[:], start=True, stop=True)
                    nc.vector.tensor_add(out=offs[:, kk * E:(kk + 1) * E],
                                         in0=offs[:, kk * E:(kk + 1) * E], in1=tot[:])
                    s = spool.tile([P, E], f32, tag=f"s{kk}")
                    nc.vector.tensor_sub(out=s[:], in0=cum[:], in1=m[:])
                    nc.vector.tensor_scalar(s[:], s[:], 1.0, None, op0=mybir.AluOpType.add)
                    nc.vector.tensor_mul(out=s[:], in0=s[:], in1=m[:])
                    dst = s0_out if kk == 0 else s1_out
                    nc.sync.dma_start(out=dst.ap()[t * P:(t + 1) * P, :], in_=s[:])
    nc.compile()
    return nc


def _build_phase_b(cap=CAP):
    """Scatter expansion: dispatch/combine have <=2 nonzero (t,e) rows per
    token; build only those 2048 rows each and indirect-scatter them into the
    pre-zeroed outputs (4x fewer bytes + 4x less DVE than a dense write)."""
    import concourse.bass as bass
    i32 = mybir.dt.int32
    NR = 2 * TOK          # (token x choice) rows per core
    NG = NR // P          # 16 scatter groups of 128 rows
    nc = bacc.Bacc("TRN2", target_bir_lowering=False, debug=False, num_devices=NCORES)
    slot = nc.dram_tensor("slot", [NR, 1], f32, kind="ExternalInput")
    prob = nc.dram_tensor("prob", [NR, 1], f32, kind="ExternalInput")
    ridx = nc.dram_tensor("ridx", [NR, 1], i32, kind="ExternalInput")
    iota_cap = nc.dram_tensor("iota_cap", [P, cap], f32, kind="ExternalInput")
    disp = nc.dram_tensor("disp", [TOK * E, cap], f32, kind="ExternalOutput")
    comb = nc.dram_tensor("comb", [TOK * E, cap], f32, kind="ExternalOutput")

    with TileContext(nc) as tc:
        with (
            tc.tile_pool(name="const", bufs=1) as cpool,
            tc.tile_pool(name="work", bufs=4) as spool,
        ):
            iota_sb = cpool.tile([P, cap], f32, tag="iota")
            nc.sync.dma_start(out=iota_sb[:], in_=iota_cap.ap()[:, :])
            # batched scatter inputs: [NR,1] -> [128, NG] (group-major columns)
            sl = cpool.tile([P, NG], f32, tag="sl")
            nc.sync.dma_start(out=sl[:], in_=slot.ap()[:, 0].rearrange("(g p) -> p g", p=P))
            pr = cpool.tile([P, NG], f32, tag="pr")
            nc.sync.dma_start(out=pr[:], in_=prob.ap()[:, 0].rearrange("(g p) -> p g", p=P))
            ri = cpool.tile([P, NG], i32, tag="ri")
            nc.sync.dma_start(out=ri[:], in_=ridx.ap()[:, 0].rearrange("(g p) -> p g", p=P))
            for g in range(NG):
                drow = spool.tile([P, cap], f32, tag="drow")
                nc.vector.tensor_scalar(drow[:], iota_sb[:], sl[:, g:g + 1], None,
                                        op0=mybir.AluOpType.is_equal)
                crow = spool.tile([P, cap], f32, tag="crow")
                nc.vector.tensor_scalar(crow[:], iota_sb[:], sl[:, g:g + 1], pr[:, g:g + 1],
                                        op0=mybir.AluOpType.is_equal,
                                        op1=mybir.AluOpType.mult)
                nc.gpsimd.indirect_dma_start(
                    out=disp.ap()[:, :],
                    out_offset=bass.IndirectOffsetOnAxis(ap=ri[:, g:g + 1], axis=0),
                    in_=drow[:], in_offset=None)
                nc.gpsimd.indirect_dma_start(
                    out=comb.ap()[:, :],
                    out_offset=bass.IndirectOffsetOnAxis(ap=ri[:, g:g + 1], axis=0),
                    in_=crow[:], in_offset=None)
    nc.compile()
    return nc


def _get(name, builder):
    if name not in _cache:
        _cache[name] = builder()
    return _cache[name]


def _split_bf16(a):
    import ml_dtypes
    hi = a.astype(ml_dtypes.bfloat16)
    lo = (a - hi.astype(np.float32)).astype(ml_dtypes.bfloat16)
    return hi, lo


def kernel(token_inputs, bottleneck_weights, expert_capacity):
    x = np.ascontiguousarray(np.asarray(token_inputs, dtype=np.float32)).reshape(B * N, D)
    w = np.ascontiguousarray(np.asarray(bottleneck_weights, dtype=np.float32))
    cap = int(expert_capacity)
    assert cap > 0

    wh, wl = _split_bf16(w)
    core_ids = list(range(NCORES))
    consts = {
        "linc": (np.arange(P)[:, None] <= np.arange(P)[None, :]).astype(np.float32),
        "ones_k1": np.ones((1, P), np.float32),
        "ones128": np.ones((P, 1), np.float32),
        "iota8": np.tile(np.arange(E, dtype=np.float32), (P, 1)),
        "ebias": np.tile(-1e-4 * np.arange(E, dtype=np.float32), (P, 1)),
    }
    in_maps_a = []
    for c in core_ids:
        shard_t = np.ascontiguousarray(x[c * TOK:(c + 1) * TOK].T)   # [2048, 1024]
        xh, xl = _split_bf16(shard_t)
        in_maps_a.append({"xTh": xh, "xTl": xl, "wh": wh, "wl": wl, **consts})

    global LAST_IN_MAPS_A, LAST_IN_MAPS_B
    LAST_IN_MAPS_A = in_maps_a
    nc_a = _get("a", _build_phase_a)
    res_a = run_bass_kernel_spmd(nc_a, in_maps_a, core_ids)

    # ---- host glue: cross-core priority offsets (16 scalars per core pair),
    # then per-(token, choice) slot / prob / target-row tables for the scatter.
    ar = np.arange(TOK)
    in_maps_b = []
    iota_cap = np.tile(np.arange(cap, dtype=np.float32), (P, 1))
    for b in range(B):
        ra, rb = res_a.results[2 * b], res_a.results[2 * b + 1]
        s0a, s1a, s0b, s1b = ra["s0"], ra["s1"], rb["s0"], rb["s1"]
        t0a = (s0a > 0).sum(0).astype(np.float32)   # [E] first-choice counts, first half
        t0b = (s0b > 0).sum(0).astype(np.float32)
        t1a = (s1a > 0).sum(0).astype(np.float32)
        n0 = t0a + t0b                               # total first-choice counts
        for s0, s1, pp, off0, off1 in (
            (s0a, s1a, ra["probs"], np.zeros(E, np.float32), n0),
            (s0b, s1b, rb["probs"], t0a, n0 + t1a),
        ):
            e0 = np.argmax(s0 > 0, axis=1)           # chosen expert per (token, k)
            e1 = np.argmax(s1 > 0, axis=1)
            slot0 = s0[ar, e0] - 1 + off0[e0]        # capacity slot (may be >= CAP)
            slot1 = s1[ar, e1] - 1 + off1[e1]
            in_maps_b.append({
                "slot": np.concatenate([slot0, slot1]).astype(np.float32)[:, None],
                "prob": np.concatenate([pp[ar, e0], pp[ar, e1]]).astype(np.float32)[:, None],
                "ridx": np.concatenate([ar * E + e0, ar * E + e1]).astype(np.int32)[:, None],
                "iota_cap": iota_cap,
            })

    LAST_IN_MAPS_B = in_maps_b
    nc_b = _get(f"b{cap}", lambda: _build_phase_b(cap))
    res_b = run_bass_kernel_spmd(nc_b, in_maps_b, core_ids)

    out = np.empty((2, B, N, E, cap), np.float32)
    for c in core_ids:
        b, h = c // 2, c % 2
        sl = slice(h * TOK, (h + 1) * TOK)
        out[0, b, sl] = res_b.results[c]["disp"].reshape(TOK, E, cap)
        out[1, b, sl] = res_b.results[c]["comb"].reshape(TOK, E, cap)
    return out



# revision 3
# speedup vs baseline: 2.6028x; 2.6028x over previous
"""MoE router (AutonomousRouter) for TRN2, 8 NeuronCores.

Computes reference:
    act    = einsum('bnd,edc->bnec', x, W)          B,N,D,E,C = 4,2048,2048,8,512
    logits = ||act||_2 over c                       [B,N,E]
    probs  = softmax(logits, -1)
    top-2 routing with capacity 640 (priority = order within k-major (choice, token) sequence)
    out    = stack([dispatch, combine])             [2,B,N,E,640] fp32

Sharding: data-parallel over tokens; core i <- tokens [i*1024, (i+1)*1024) of the
flattened [8192] token axis (= batch b=i//2, half i%2). Weights replicated.

Phase A (device): single-pass bf16 matmuls -> sum-of-squares per (token, expert).
Host glue: routing control. bf16 rounding perturbs sumsq by <~0.45 abs (scale ~420);
tokens whose top-3 sumsq gaps fall inside a 1.0 margin (~10-13%) are recomputed in
fp32 on host so the top-2 selection/order matches the fp32 reference exactly.
Softmax, top-2, and the k-major capacity cumsum are tiny [8192,8] numpy ops.
Phase B (device): dispatch/combine have <=2 nonzero scalars per (token, expert)
row; scatter exactly those elements (value 1.0 resp. prob) into the pre-zeroed
dense outputs via per-element indirect DMA. Capacity-overflow entries are
redirected to a spill tail that the host slices off.
"""
import numpy as np

import concourse.bacc as bacc
import concourse.mybir as mybir
from concourse.tile import TileContext
from concourse.bass_utils import run_bass_kernel_spmd

P = 128          # partitions
B, N, D, E, C = 4, 2048, 2048, 8, 512
CAP = 640
NCORES = 8
TOK = (B * N) // NCORES     # tokens per core = 1024
NT = TOK // P               # token tiles per core = 8
KT = D // P                 # contraction tiles = 16
MARGIN = 1.0                # sumsq gap below which host recomputes in fp32

f32 = mybir.dt.float32

_cache = {}
LAST_IN_MAPS_A = None   # kept for test harness re-runs/profiling
LAST_IN_MAPS_B = None


def _build_phase_a():
    bf16 = mybir.dt.bfloat16
    nc = bacc.Bacc("TRN2", target_bir_lowering=False, debug=False, num_devices=NCORES)
    xT = nc.dram_tensor("xT", [D, TOK], bf16, kind="ExternalInput")
    w = nc.dram_tensor("w", [E, D, C], bf16, kind="ExternalInput")
    ss_out = nc.dram_tensor("ss", [TOK, E], f32, kind="ExternalOutput")

    with TileContext(nc) as tc:
        with (
            tc.tile_pool(name="const", bufs=1) as cpool,
            tc.tile_pool(name="wbuf", bufs=2) as wpool,
            tc.tile_pool(name="work", bufs=3) as spool,
            tc.tile_pool(name="psum", bufs=8, space="PSUM") as psum,
        ):
            # x^T resident in variable k-chunk tiles; W per expert likewise
            # (double-buffered). DMAs are issued in consumption order and the
            # first chunk is a single k-block, so the first matmuls wait on
            # ~0.4MB instead of the full 20MB.
            CHUNKS = [1, 3, 4, 4, 4]           # k-blocks per chunk, sums to KT
            CH0 = [sum(CHUNKS[:i]) for i in range(len(CHUNKS))]  # chunk k-starts
            NCH = len(CHUNKS)

            def _x_chunk(q):
                nk = CHUNKS[q]
                name = f"xq{q}"
                tile_ = cpool.tile([P, nk * TOK], bf16, tag=name, name=name)
                nc.sync.dma_start(
                    out=tile_[:].rearrange("p (k n) -> p k n", k=nk),
                    in_=xT.ap()[CH0[q] * P:(CH0[q] + nk) * P, :]
                        .rearrange("(k p) n -> p k n", p=P),
                )
                return tile_

            def _w_chunk(e, q):
                nk = CHUNKS[q]
                tile_ = wpool.tile([P, nk * C], bf16, tag=f"wq{q}", name=f"w{e}_{q}")
                nc.sync.dma_start(
                    out=tile_[:].rearrange("p (k c) -> p k c", k=nk),
                    in_=w.ap()[e, CH0[q] * P:(CH0[q] + nk) * P, :]
                        .rearrange("(k p) c -> p k c", p=P),
                )
                return tile_

            # consumption-order issue: W(e0,q0), x(q0), W(e0,q1), x(q1), ...
            w0_q, x_q = [], []
            for q in range(NCH):
                w0_q.append(_w_chunk(0, q))
                x_q.append(_x_chunk(q))

            # per-token-tile sum-of-squares accumulators [128, E]
            ss_tiles = [cpool.tile([P, E], f32, tag=f"ss{t}", name=f"ss{t}")
                        for t in range(NT)]

            # ---- matmul phase: for each expert, 8 token tiles x 16 k-tiles ----
            for e in range(E):
                w_q = w0_q if e == 0 else [_w_chunk(e, q) for q in range(NCH)]
                for t in range(NT):
                    ps = psum.tile([P, C], f32, space="PSUM", tag="ps")
                    for k in range(KT):
                        q = max(i for i in range(NCH) if CH0[i] <= k)
                        kq = k - CH0[q]
                        x_blk = x_q[q][:, kq * TOK + t * P: kq * TOK + (t + 1) * P]
                        w_blk = w_q[q][:, kq * C:(kq + 1) * C]
                        nc.tensor.matmul(ps[:], lhsT=x_blk, rhs=w_blk,
                                         start=(k == 0), stop=(k == KT - 1))
                    sq = spool.tile([P, C], f32, tag="sq")
                    nc.scalar.activation(sq[:], ps[:], mybir.ActivationFunctionType.Square)
                    red8 = spool.tile([P, 8], f32, tag="red8")
                    nc.vector.tensor_reduce(
                        red8[:], sq[:].rearrange("p (g c) -> p g c", g=8),
                        axis=mybir.AxisListType.X, op=mybir.AluOpType.add,
                    )
                    nc.vector.tensor_reduce(
                        ss_tiles[t][:, e:e + 1], red8[:],
                        axis=mybir.AxisListType.X, op=mybir.AluOpType.add,
                    )
            for t in range(NT):
                nc.sync.dma_start(out=ss_out.ap()[t * P:(t + 1) * P, :], in_=ss_tiles[t][:])
    nc.compile()
    return nc


def _build_phase_b(cap=CAP):
    """Element scatter: each (token, choice) contributes exactly one nonzero
    scalar to dispatch (1.0) and combine (prob) at flat index
    (t*E + e)*cap + slot. Scatter 4096 single floats via indirect DMA —
    no one-hot row materialization, ~16KB written."""
    import concourse.bass as bass
    i32 = mybir.dt.int32
    NR = 2 * TOK          # (token x choice) rows per core
    NG = NR // P          # 16 scatter groups of 128 elements
    SPILL = NR            # capacity-overflow elements land past the real output
    nc = bacc.Bacc("TRN2", target_bir_lowering=False, debug=False, num_devices=NCORES)
    fidx = nc.dram_tensor("fidx", [NR, 1], i32, kind="ExternalInput")
    pval = nc.dram_tensor("pval", [NR, 1], f32, kind="ExternalInput")
    disp = nc.dram_tensor("disp", [TOK * E * cap + SPILL, 1], f32, kind="ExternalOutput")
    comb = nc.dram_tensor("comb", [TOK * E * cap + SPILL, 1], f32, kind="ExternalOutput")

    with TileContext(nc) as tc:
        with tc.tile_pool(name="const", bufs=1) as cpool:
            fi = cpool.tile([P, NG], i32, tag="fi")
            nc.sync.dma_start(out=fi[:], in_=fidx.ap()[:, 0].rearrange("(g p) -> p g", p=P))
            pv = cpool.tile([P, NG], f32, tag="pv")
            nc.sync.dma_start(out=pv[:], in_=pval.ap()[:, 0].rearrange("(g p) -> p g", p=P))
            on = cpool.tile([P, 1], f32, tag="on")
            nc.vector.memset(on[:], 1.0)
            for g in range(NG):
                nc.gpsimd.indirect_dma_start(
                    out=disp.ap()[:, :],
                    out_offset=bass.IndirectOffsetOnAxis(ap=fi[:, g:g + 1], axis=0),
                    in_=on[:], in_offset=None)
                nc.gpsimd.indirect_dma_start(
                    out=comb.ap()[:, :],
                    out_offset=bass.IndirectOffsetOnAxis(ap=fi[:, g:g + 1], axis=0),
                    in_=pv[:, g:g + 1], in_offset=None)
    nc.compile()
    return nc


def _get(name, builder):
    if name not in _cache:
        _cache[name] = builder()
    return _cache[name]


def kernel(token_inputs, bottleneck_weights, expert_capacity):
    import ml_dtypes
    x = np.ascontiguousarray(np.asarray(token_inputs, dtype=np.float32)).reshape(B * N, D)
    w = np.ascontiguousarray(np.asarray(bottleneck_weights, dtype=np.float32))
    cap = int(expert_capacity)
    assert cap > 0

    wb = w.astype(ml_dtypes.bfloat16)
    core_ids = list(range(NCORES))
    in_maps_a = []
    for c in core_ids:
        shard_t = np.ascontiguousarray(x[c * TOK:(c + 1) * TOK].T)   # [2048, 1024]
        in_maps_a.append({"xT": shard_t.astype(ml_dtypes.bfloat16), "w": wb})

    global LAST_IN_MAPS_A, LAST_IN_MAPS_B
    LAST_IN_MAPS_A = in_maps_a
    nc_a = _get("a", _build_phase_a)
    res_a = run_bass_kernel_spmd(nc_a, in_maps_a, core_ids)

    # ---- host glue: routing control on [8192, 8] scalars ----
    ss = np.concatenate([res_a.results[c]["ss"] for c in core_ids], axis=0)  # [8192, 8]

    # fp64 recompute of tokens whose bf16-grade sumsq cannot certify the
    # reference's fp32 top-2 selection/order (top-3 gaps inside MARGIN)
    ss = ss.astype(np.float64)
    srt = np.sort(ss, axis=1)
    flag = ((srt[:, -1] - srt[:, -2]) < MARGIN) | ((srt[:, -2] - srt[:, -3]) < MARGIN)
    if flag.any():
        xf = x[flag].astype(np.float64)
        for e in range(E):
            act = xf @ w[e].astype(np.float64)
            ss[flag, e] = np.einsum('tc,tc->t', act, act)

    logits = np.sqrt(ss)
    ex = np.exp(logits - logits.max(axis=1, keepdims=True))
    probs = ex / ex.sum(axis=1, keepdims=True)            # float64, monotone in ss
    # selection on ss - e*1e-4: sub-ulp lower-index bias so fp32-noise-level
    # near-ties resolve the way the reference's fp32 top_k resolves them
    order = np.argsort(-(ss - 1e-4 * np.arange(E)), axis=1, kind='stable')
    e0g, e1g = order[:, 0], order[:, 1]

    ar = np.arange(TOK)
    base = TOK * E * cap
    in_maps_b = []
    for b in range(B):
        sl = slice(b * N, (b + 1) * N)
        e0b, e1b = e0g[sl], e1g[sl]
        seq = np.concatenate([e0b, e1b])                  # k-major (choice, token)
        slots = np.empty(2 * N, np.int64)
        for e in range(E):
            m = seq == e
            slots[m] = np.arange(m.sum())
        pb = probs[sl]
        for h in range(2):
            tg = h * TOK + ar                             # batch-local token idx
            s0, s1 = slots[:N][tg], slots[N:][tg]
            ee0, ee1 = e0b[tg], e1b[tg]
            fi0 = np.where(s0 < cap, (ar * E + ee0) * cap + s0, base + ar)
            fi1 = np.where(s1 < cap, (ar * E + ee1) * cap + s1, base + TOK + ar)
            in_maps_b.append({
                "fidx": np.concatenate([fi0, fi1]).astype(np.int32)[:, None],
                "pval": np.concatenate([pb[tg, ee0], pb[tg, ee1]]).astype(np.float32)[:, None],
            })

    LAST_IN_MAPS_B = in_maps_b
    nc_b = _get(f"b{cap}", lambda: _build_phase_b(cap))
    res_b = run_bass_kernel_spmd(nc_b, in_maps_b, core_ids)

    out = np.empty((2, B, N, E, cap), np.float32)
    for c in core_ids:
        b, h = c // 2, c % 2
        sl = slice(h * TOK, (h + 1) * TOK)
        out[0, b, sl] = res_b.results[c]["disp"].reshape(-1)[:base].reshape(TOK, E, cap)
        out[1, b, sl] = res_b.results[c]["comb"].reshape(-1)[:base].reshape(TOK, E, cap)
    return out
